# revision 27
# baseline (speedup 1.0000x reference)
"""GQA forward (b=2, s=2048, H=32 q heads, 8 kv heads, d=64) on 8 TRN2 cores.

Sharding: core k owns query heads 4k..4k+3 and kv head k. GQA group
structure makes attention fully local per core (q heads 4k..4k+3 attend
only to kv head k). x is replicated; W columns are sharded; outputs are
column-concatenated.

Per-core kernel (Tile framework), fp16 datapath / fp32 accumulation:
  - x.T is produced on the HOST (numpy transpose + fp16 cast) and DMA'd
    straight into SBUF — no on-chip transposes of x.
  - Projections in natural layout: QKV[s,384] = xT_chunk.T @ W_chunk
    accumulated in fp32 PSUM over 16 k-chunks (fp16 operands, 1 cyc/row).
  - RoPE on DVE with free-dim stride-2 views, fused with the PSUM->SBUF
    eviction (sin table pre-negated on host so plain tensor_tensor
    suffices); V columns go straight into the [V|1] resident.
  - Q/K flipped to [d, s] via PE transposes (fp16, 1 cyc/row).
  - Attention in transposed layout: S.T[kv,q] = K @ Q.T per 128-kv block,
    exp on ACT (scale=1/8 folded in) with fp16 output, causal handled by
    skipping blocks above the diagonal + multiplying the diagonal block
    of P by a 0/1 fp16 mask, ctx.T[65,q] = [V|1].T @ P.T accumulated in
    fp32 PSUM (row 64 = softmax sums).
  - Finalize: 4 PE transposes of ctx.T into one [128,4,66] PSUM bank,
    one reciprocal, 4 scalar muls, one DMA per (head, s-tile).
"""

import numpy as np
from contextlib import ExitStack

import concourse.bass as bass
import concourse.bacc as bacc
import concourse.mybir as mybir
from concourse import tile
from concourse.bass_utils import run_bass_kernel_spmd

F32 = mybir.dt.float32
F16 = mybir.dt.float16
MUL = mybir.AluOpType.mult
ADD = mybir.AluOpType.add

B = 2
S = 2048
DIN = 2048
D = 64              # head dim
HPC = 4             # query heads per core
NCORES = 8
WCOLS = 4 * D + D + D  # 256 q cols + 64 k + 64 v = 384
RC = 320            # roped columns (4 q heads + k head)
ST = 512            # s-tile (rows per outer step)
NST = B * S // ST   # 8 s-tiles
NCH = DIN // 128    # 16 k-chunks
NKV = S // 128      # kv tiles per batch


def build_bass():
    nc = bacc.Bacc(None, target_bir_lowering=False)
    xt_d = nc.declare_dram_parameter("xt", [DIN, B * S], F16, isOutput=False)
    w_d = nc.declare_dram_parameter("w", [DIN, WCOLS], F16, isOutput=False)
    cos_d = nc.declare_dram_parameter("cosn", [S, RC], F16, isOutput=False)
    sin_d = nc.declare_dram_parameter("sinn", [S, RC], F16, isOutput=False)
    mask_d = nc.declare_dram_parameter("mask", [128, 128], F16, isOutput=False)
    id16_d = nc.declare_dram_parameter("id16", [128, 128], F16, isOutput=False)
    id32_d = nc.declare_dram_parameter("id32", [128, 128], F32, isOutput=False)
    out_d = nc.declare_dram_parameter("out", [B * S, HPC * D], F32, isOutput=True)

    with ExitStack() as ctx:
        tc = ctx.enter_context(tile.TileContext(nc))
        const = ctx.enter_context(tc.tile_pool(name="const", bufs=1))
        resid = ctx.enter_context(tc.tile_pool(name="resid", bufs=1))
        xt_p = ctx.enter_context(tc.tile_pool(name="xt", bufs=2))
        qn_p = ctx.enter_context(tc.tile_pool(name="qn", bufs=3))
        qt_p = ctx.enter_context(tc.tile_pool(name="qt", bufs=4))
        p_p = ctx.enter_context(tc.tile_pool(name="p", bufs=4))
        cx_p = ctx.enter_context(tc.tile_pool(name="cx", bufs=2))
        o_p = ctx.enter_context(tc.tile_pool(name="o", bufs=3))
        rv_p = ctx.enter_context(tc.tile_pool(name="rv", bufs=3))
        tp_ps = ctx.enter_context(tc.tile_pool(name="tp_ps", bufs=2, space="PSUM"))
        pr_ps = ctx.enter_context(tc.tile_pool(name="pr_ps", bufs=2, space="PSUM"))
        sc_ps = ctx.enter_context(tc.tile_pool(name="sc_ps", bufs=2, space="PSUM"))
        cx_ps = ctx.enter_context(tc.tile_pool(name="cx_ps", bufs=2, space="PSUM"))

        # constants on the SCALAR engine's DMA queue so the x.T tiles (sync
        # queue) stream in parallel — the first projection only waits for
        # w chunk-group 0 + xt chunk-group 0 instead of ~7MB of constants.
        w_sb = const.tile([128, NCH, WCOLS], F16)
        for cg in range(4):
            nc.scalar.dma_start(
                out=w_sb[:, cg * 4:(cg + 1) * 4, :],
                in_=w_d[cg * 512:(cg + 1) * 512, :].rearrange(
                    "(c p) n -> p c n", p=128))
        mask_sb = const.tile([128, 128], F16)
        nc.scalar.dma_start(out=mask_sb[:], in_=mask_d[:])
        id16 = const.tile([128, 128], F16)
        nc.scalar.dma_start(out=id16[:], in_=id16_d[:])
        id32 = const.tile([128, 128], F32)
        nc.scalar.dma_start(out=id32[:], in_=id32_d[:])
        # rope tables resident, t-major ([128, t, col]); both batches share.
        # chunk-group cg holds t=4cg..4cg+3, exactly what s-tile cg%4 needs.
        ctab = const.tile([128, NCH, RC], F16)
        stab = const.tile([128, NCH, RC], F16)
        for cg in range(4):
            nc.scalar.dma_start(
                out=ctab[:, cg * 4:(cg + 1) * 4, :],
                in_=cos_d[cg * 512:(cg + 1) * 512, :].rearrange(
                    "(t p) n -> p t n", p=128))
            nc.scalar.dma_start(
                out=stab[:, cg * 4:(cg + 1) * 4, :],
                in_=sin_d[cg * 512:(cg + 1) * 512, :].rearrange(
                    "(t p) n -> p t n", p=128))

        # rows 0-63: K.T (RoPE'd); rows 64-127: duplicate copy so that the
        # scores matmul lhsT can match either base partition of the Q halves
        kt_res = resid.tile([128, B * S], F16)
        vp_res = resid.tile([128, B * NKV, 128], F16)  # [V|1|0pad] kv-tiles
        nc.vector.memset(vp_res[:], 0.0)
        nc.vector.memset(vp_res[:, :, 64:65], 1.0)

        for st in range(NST):
            b, sti = divmod(st, 4)

            # ---- x.T tile straight from HBM (host-transposed), split so the
            # first chunk-group's projections can start before the rest land
            xt = xt_p.tile([128, NCH, ST], F16, tag="xt")
            for cg in range(4):
                nc.sync.dma_start(
                    out=xt[:, cg * 4:(cg + 1) * 4, :],
                    in_=xt_d[cg * 512:(cg + 1) * 512,
                             st * ST:(st + 1) * ST].rearrange(
                                 "(c p) s -> p c s", p=128))

            # ---- projections (natural layout) + RoPE + transposes ----
            qta = qt_p.tile([128, ST], F16, tag="qta")   # heads 0,1 as [d,s]
            qtb = qt_p.tile([128, ST], F16, tag="qtb")   # heads 2,3 as [d,s]
            for pt in range(4):
                t = sti * 4 + pt  # within-batch 128-row block index
                pp = pr_ps.tile([128, WCOLS], F32, tag="pp")
                for c in range(NCH):
                    nc.tensor.matmul(
                        pp[:], xt[:, c, pt * 128:(pt + 1) * 128],
                        w_sb[:, c, :], start=(c == 0), stop=(c == NCH - 1))
                qn = qn_p.tile([128, RC], F16, tag="qn")
                ts = qn_p.tile([128, RC], F32, tag="ts")
                # even cols: qe*c - qo*s ; odd cols: qo*c + qe*s
                # (sin table pre-negated on host in even columns)
                nc.vector.tensor_tensor(
                    ts[:, 0:RC:2], pp[:, 1:RC:2], stab[:, t, 0:RC:2], MUL)
                nc.vector.tensor_tensor(
                    ts[:, 1:RC:2], pp[:, 0:RC:2], stab[:, t, 1:RC:2], MUL)
                nc.vector.tensor_tensor(qn[:], pp[:, 0:RC], ctab[:, t, :], MUL)
                nc.vector.tensor_tensor(qn[:], qn[:], ts[:], ADD)
                # V columns: straight into the [V|1] resident ([kv, d] natural)
                nc.vector.tensor_copy(
                    vp_res[:, b * NKV + t, 0:64], pp[:, RC:WCOLS])
                # flip Q to [d, s]
                for cb in range(2):
                    tp = tp_ps.tile([128, 128], F16, tag="tp")
                    nc.tensor.transpose(
                        tp[:], qn[:, cb * 128:(cb + 1) * 128], id16[:])
                    dst = qta if cb == 0 else qtb
                    nc.vector.tensor_copy(
                        dst[:, pt * 128:(pt + 1) * 128], tp[:])
                # flip K ([128, 64] -> [64, 128])
                tpk = tp_ps.tile([128, 128], F16, tag="tp")
                nc.tensor.transpose(tpk[0:64, :], qn[:, 256:320], id16[:])
                nc.vector.tensor_copy(
                    kt_res[0:64, st * ST + pt * 128:st * ST + (pt + 1) * 128],
                    tpk[0:64, :])
            nc.gpsimd.dma_start(
                out=kt_res[64:128, st * ST:(st + 1) * ST],
                in_=kt_res[0:64, st * ST:(st + 1) * ST])

            # ---- attention for the 4 heads of this q-tile ----
            # full kv blocks first, diagonal blocks (which need the extra
            # mask op between exp and ctx) last; js[0] is always w0=0 so the
            # start=True ctx matmul initializes the whole bank.
            js = list(range(4 * sti)) + \
                 [4 * sti, 4 * sti + 1, 4 * sti + 2, 4 * sti + 3]
            for h in range(HPC):
                p0 = (h % 2) * 64
                qh = (qta if h < 2 else qtb)[p0:p0 + 64, :]
                cxt = cx_ps.tile([128, ST], F32, tag="cxt")
                # each ctx matmul is emitted one step behind its scores so
                # the next scores matmul isn't queued behind a ctx that is
                # still waiting on its exp (the PE executes in order)
                pend = None  # (psb, w0, start, vp slot)
                for idx, j in enumerate(js):
                    off = 128 * j - 512 * sti
                    w0 = max(0, off)
                    sc = sc_ps.tile([128, ST], F32, tag="sc")
                    nc.tensor.matmul(
                        sc[:, w0:ST],
                        kt_res[p0:p0 + 64, b * S + j * 128:b * S + (j + 1) * 128],
                        qh[:, w0:ST], start=True, stop=True)
                    psb = p_p.tile([128, ST], F16, tag="psb")
                    nc.scalar.activation(
                        psb[:, w0:ST], sc[:, w0:ST],
                        mybir.ActivationFunctionType.Exp, scale=0.125)
                    if j >= 4 * sti:
                        # zero the upper-triangle of the diagonal block
                        nc.vector.tensor_tensor(
                            psb[:, off:off + 128], psb[:, off:off + 128],
                            mask_sb[:], MUL)
                    if pend is not None:
                        nc.tensor.matmul(
                            cxt[:, pend[1]:ST], vp_res[:, pend[3], :],
                            pend[0][:, pend[1]:ST], start=pend[2], stop=False)
                    pend = (psb, w0, idx == 0, b * NKV + j)
                nc.tensor.matmul(
                    cxt[:, pend[1]:ST], vp_res[:, pend[3], :],
                    pend[0][:, pend[1]:ST], start=pend[2], stop=True)
                cxs = cx_p.tile([65, ST], F32, tag="cxs")
                nc.vector.tensor_copy(cxs[:], cxt[0:65, :])
                # fi shares the cx_ps buffers (same tag/shape as cxt): ctx of
                # head h+1 reuses the buffer fi of head h-1 released
                fi = cx_ps.tile([128, ST], F32, tag="cxt")
                for qq in range(4):
                    nc.tensor.transpose(
                        fi[:, qq * 128:qq * 128 + 66],
                        cxs[:, qq * 128:(qq + 1) * 128],
                        id32[0:65, 0:66])
                rv = rv_p.tile([128, 4], F32, tag="rv")
                nc.vector.reciprocal(rv[:], fi[:, 64:ST:128])
                ob = o_p.tile([128, 4, 64], F32, tag="ob")
                for qq in range(4):
                    nc.vector.tensor_scalar_mul(
                        ob[:, qq, :], fi[:, qq * 128:qq * 128 + 64],
                        rv[:, qq:qq + 1])
                nc.gpsimd.dma_start(
                    out=out_d[st * ST:(st + 1) * ST,
                              h * 64:(h + 1) * 64].rearrange(
                                  "(q p) d -> p q d", p=128),
                    in_=ob[:])
    return nc


_NC_CACHE = None


def _host_consts():
    i = np.arange(0, D, 2, dtype=np.float64) / D          # 32 pair exponents
    freqs = 1.0 / (10000.0 ** i)                           # (32,)
    ang = np.arange(S, dtype=np.float64)[:, None] * freqs[None, :]  # (S, 32)
    cos = np.cos(ang).astype(np.float32)                   # (S, 32)
    sin = np.sin(ang).astype(np.float32)
    dcol = (np.arange(RC) % D) // 2                        # (320,) pair idx
    sinn = np.ascontiguousarray(sin[:, dcol])
    sinn[:, 0::2] *= -1.0                                  # pre-negate evens
    cosn = np.ascontiguousarray(cos[:, dcol]).astype(np.float16)  # (S, 320)
    sinn = sinn.astype(np.float16)
    kv, qq = np.meshgrid(np.arange(128), np.arange(128), indexing="ij")
    mask01 = (kv <= qq).astype(np.float16)                 # 1 = allowed
    ident16 = np.eye(128, dtype=np.float16)
    ident32 = np.eye(128, dtype=np.float32)
    return cosn, sinn, mask01, ident16, ident32


def _in_maps(x, Wq, Wk, Wv):
    x = np.asarray(x, dtype=np.float32).reshape(B * S, DIN)
    xt = np.ascontiguousarray(x.T).astype(np.float16)      # [DIN, B*S]
    Wq = np.asarray(Wq, dtype=np.float32)
    Wk = np.asarray(Wk, dtype=np.float32)
    Wv = np.asarray(Wv, dtype=np.float32)
    cosn, sinn, mask01, ident16, ident32 = _host_consts()

    in_maps = []
    for k in range(NCORES):
        w_all = np.hstack([
            Wq[:, k * 256:(k + 1) * 256],
            Wk[:, k * 64:(k + 1) * 64],
            Wv[:, k * 64:(k + 1) * 64],
        ]).astype(np.float16)
        in_maps.append({
            "xt": xt, "w": np.ascontiguousarray(w_all),
            "cosn": cosn, "sinn": sinn, "mask": mask01,
            "id16": ident16, "id32": ident32,
        })
    return in_maps


def _run(in_maps, **kwargs):
    global _NC_CACHE
    if _NC_CACHE is None:
        _NC_CACHE = build_bass()
        _NC_CACHE.finalize()
    return run_bass_kernel_spmd(_NC_CACHE, in_maps, list(range(NCORES)),
                                **kwargs)


def kernel(x, Wq, Wk, Wv):
    res = _run(_in_maps(x, Wq, Wk, Wv))
    out = np.concatenate([res.results[k]["out"] for k in range(NCORES)], axis=1)
    return out.reshape(B, S, 32 * D)
